# revision 13
# baseline (speedup 1.0000x reference)
"""GQA causal attention block (x @ Wq/Wk/Wv -> causal GQA attention -> @ Wo)
for Trainium2, SPMD over 8 NeuronCores.

Sharding: 4 batches x 2 query-shards. Core c handles batch c//2 and the
interleaved set of 128-row query tiles {s, s+2, ...} (s = c%2), which
balances the causal-attention triangle between the two shards of a batch.

vs. the v1 kernel:
- k/v are projected only for the core's own rows; the two cores of a
  batch swap halves through two 8-core AllGathers (one per 512-row
  chunk, launched as soon as that chunk's k/v is ready, running on
  TOPSP/SDMA fully overlapped with q-projection).
- x is transposed once (own rows only) and reused for q-projection.
- Wq is streamed once per head-group, Wo once; o-projection runs in a
  single pass using all 8 PSUM banks.
- softmax reciprocals use the fast approximate DVE op (~5x cheaper).

The attention inner loop keeps the dense 3-matmul form (scores,
ones-rowsum, p@V) of v1: the PE clock throttles down when its duty
cycle drops, so "saving" the rowsum matmul makes everything slower.

Key-slot layout keeps the SPMD program shard-independent: slots 0..7
hold the core's own key tiles (local order), slots 8..15 the
partner's. Query tile j attends over slots {0..j} u {8..8+j}; the
host-provided masks make it causal: masks[0] (slot j, the own-side
diagonal) is triangular for both shards, masks[1] (slot 8+j) is -inf
for shard 0 (future keys) and 0 for shard 1 (past keys). The partner
block's position in the AllGather output is the only rank-dependent
address, supplied per-core as a uint32 element offset ("poff") and
used as a runtime DMA offset register.
"""

import sys

for _p in ("/opt/trn_rl_repo", "/root/.axon_site/_ro/trn_rl_repo"):
    if _p not in sys.path:
        sys.path.append(_p)

import numpy as np
import ml_dtypes

import concourse.bacc as bacc
import concourse.bass as bass
import concourse.tile as tile
import concourse.mybir as mybir
from concourse.bass_utils import run_bass_kernel_spmd

F32 = mybir.dt.float32
BF16 = mybir.dt.bfloat16
U32 = mybir.dt.uint32
AF = mybir.ActivationFunctionType
NEG = -1.0e6  # additive mask for disallowed keys (pre-softmax-scale)


class Cfg:
    def __init__(self, T, E, H, KV, n_batch, n_shard):
        self.T, self.E, self.H, self.KV = T, E, H, KV
        self.D = 128
        self.G4 = H // 4             # 4-head kv groups
        self.NE = E // 128           # contraction chunks for projections
        self.n_batch = n_batch
        self.n_shard = n_shard
        self.n_cores = n_batch * n_shard
        self.RQ = T // n_shard       # query rows per core
        self.NJ = self.RQ // 128     # local 128-row query tiles
        self.NLT = self.RQ // 512    # local 512-row chunks
        self.NT = T // 128           # global 128-row tiles
        self.HKV = KV * self.D       # k/v projection width
        self.scale = 1.0 / float(np.sqrt(self.D))
        # per-chunk exchange block: kT (KV heads) + v (4 local tiles)
        self.CCB = (self.KV + 4) * 128  # rows per cc_in buffer


FULL = Cfg(T=2048, E=2048, H=16, KV=4, n_batch=4, n_shard=2)


def build(cfg):
    c = cfg
    nc = bacc.Bacc("TRN2", target_bir_lowering=False, debug=False,
                   num_devices=c.n_cores)

    xq_d = nc.dram_tensor("xq", [c.RQ, c.E], BF16, kind="ExternalInput").ap()
    wq_d = nc.dram_tensor("Wq", [c.E, c.H * c.D], BF16, kind="ExternalInput").ap()
    wk_d = nc.dram_tensor("Wk", [c.E, c.HKV], BF16, kind="ExternalInput").ap()
    wv_d = nc.dram_tensor("Wv", [c.E, c.HKV], BF16, kind="ExternalInput").ap()
    wo_d = nc.dram_tensor("Wo", [c.H * c.D, c.E], BF16, kind="ExternalInput").ap()
    mask_d = nc.dram_tensor("masks", [2, 128, 512], F32,
                            kind="ExternalInput").ap()
    idb_d = nc.dram_tensor("identb", [128, 128], BF16, kind="ExternalInput").ap()
    onesb_d = nc.dram_tensor("onesb", [128, 128], BF16, kind="ExternalInput").ap()
    poff_d = nc.dram_tensor("poff", [1, 1], U32, kind="ExternalInput").ap()
    o_d = nc.dram_tensor("o", [c.RQ, c.E], F32, kind="ExternalOutput").ap()

    from contextlib import ExitStack
    with tile.TileContext(nc) as tc:
        with ExitStack() as _st:
            def pool(name, bufs, space="SBUF"):
                return _st.enter_context(
                    tc.tile_pool(name=name, bufs=bufs, space=space))
            constp = pool("const", 1)
            xqtp = pool("xqt", c.NE)
            ktp = pool("kts", c.KV * 4)
            vp = pool("vsb", c.NT)
            qtp = pool("qt", 16)
            ytp = pool("yt", c.G4 * c.NJ)
            wqp = pool("wq", c.NE)
            wkvp = pool("wkv", 6)
            wop = pool("wo", 8)
            smp = pool("sm", 8)
            bsbp = pool("bsb", 4)
            xnp = pool("xn", 8)
            osbp = pool("osb", 6)
            pq = pool("pq", 2, space="PSUM")
            pa = pool("pa", 2, space="PSUM")
            py = pool("py", 4, space="PSUM")
            dramp = pool("dram", 1, space="DRAM")

            # --- constants (identb first: the warmup needs it; the rest go
            # on the scalar queue so they don't delay the first x tiles) ---
            identb = constp.tile([128, 128], BF16, tag="identb")
            nc.sync.dma_start(identb[:], idb_d[:])
            masks = []
            for i in range(2):
                m = constp.tile([128, 512], F32, tag=f"mask{i}", name=f"mask{i}")
                nc.scalar.dma_start(m[:], mask_d[i])
                masks.append(m)
            onesb = constp.tile([128, 128], BF16, tag="onesb")
            nc.scalar.dma_start(onesb[:], onesb_d[:])
            poffs = constp.tile([1, 1], U32, tag="poffs")
            nc.scalar.dma_start(poffs[:], poff_d[:])

            cc_in = [dramp.tile([c.CCB, 512], BF16, name=f"cc_in{lt}",
                                tag=f"cc_in{lt}") for lt in range(c.NLT)]
            cc_out = [dramp.tile([2 * c.CCB, 512], BF16,
                                 name=f"cc_out{lt}",
                                 tag=f"cc_out{lt}") for lt in range(c.NLT)]

            # warm the PE clock-gate during the initial DMA ramp
            pwu = pa.tile([128, 512], BF16, tag="pa", name="pwu")
            for wu in range(24):
                nc.tensor.transpose(pwu[:, (wu % 4) * 128:(wu % 4 + 1) * 128],
                                    identb[:], identb[:])

            # persistent activations
            xqT = [xqtp.tile([128, c.RQ], BF16, tag="xqT", name=f"xqT{e}")
                   for e in range(c.NE)]
            kts = [[ktp.tile([128, 512], BF16, tag="kts", name=f"kts{h}_{q}")
                    for q in range(4)] for h in range(c.KV)]
            v_sb = [vp.tile([128, c.HKV], BF16, tag="v", name=f"v{i}")
                    for i in range(c.NT)]

            # partner block offset (elements) comes from host data
            poff_r = nc.gpsimd.alloc_register("poff_r")
            nc.gpsimd.reg_load(poff_r, poffs[0:1, 0:1])
            poff_v = nc.gpsimd.snap(poff_r, donate=True, min_val=0,
                                    max_val=c.CCB * 512)

            def cc_src(lt, block):
                off = poff_v + block * 128 * 512
                return bass.AP(cc_out[lt].tensor, off, [[512, 128], [1, 512]])

            # ---------------- Phase A: transposes + own-half k/v ------------
            def phase_a(lt):
                # transpose own 512 rows into xqT[e][:, lt*512:(lt+1)*512]
                for qa in range(c.NE // 4):
                    xns = []
                    for i in range(4):
                        xn = xnp.tile([128, 512], BF16, tag="xn",
                                      name=f"xn{i}")
                        nc.sync.dma_start(
                            xn[:], xq_d[lt * 512 + i * 128:
                                        lt * 512 + (i + 1) * 128,
                                        qa * 512:(qa + 1) * 512])
                        xns.append(xn)
                    for eh in range(4):
                        e = qa * 4 + eh
                        ptr = pa.tile([128, 512], BF16, tag="pa", name="ptr")
                        for i in range(4):
                            nc.tensor.transpose(
                                ptr[:, i * 128:(i + 1) * 128],
                                xns[i][:, eh * 128:(eh + 1) * 128], identb[:])
                        nc.vector.tensor_copy(
                            xqT[e][:, lt * 512:(lt + 1) * 512], ptr[:])

                # kT for own rows -> slots 4*lt..4*lt+3 (= quad lt)
                psk = ([pq.tile([128, 512], F32, tag="pq", name=f"psk{h}")
                        for h in range(2)] +
                       [pa.tile([128, 512], F32, tag="pa", name=f"psk{h + 2}")
                        for h in range(2)])
                for e in range(c.NE):
                    wk_t = wkvp.tile([128, c.HKV], BF16, tag="wkv", name="wk_t")
                    nc.gpsimd.dma_start(wk_t[:], wk_d[e * 128:(e + 1) * 128, :])
                    for h in range(c.KV):
                        nc.tensor.matmul(psk[h][:],
                                         wk_t[:, h * 128:(h + 1) * 128],
                                         xqT[e][:, lt * 512:(lt + 1) * 512],
                                         start=(e == 0), stop=(e == c.NE - 1))
                for h in range(c.KV):
                    nc.vector.tensor_copy(kts[h][lt][:], psk[h][:])
                    nc.scalar.dma_start(
                        cc_in[lt][h * 128:(h + 1) * 128, :], kts[h][lt][:])

                # v for own rows -> slots 4*lt..4*lt+3
                psv = ([pq.tile([128, c.HKV], F32, tag="pq", name=f"psv{i}")
                        for i in range(2)] +
                       [pa.tile([128, c.HKV], F32, tag="pa", name=f"psv{i + 2}")
                        for i in range(2)])
                for e in range(c.NE):
                    wv_t = wkvp.tile([128, c.HKV], BF16, tag="wkv", name="wv_t")
                    nc.gpsimd.dma_start(wv_t[:], wv_d[e * 128:(e + 1) * 128, :])
                    for i in range(4):
                        nc.tensor.matmul(psv[i][:],
                                         xqT[e][:, lt * 512 + i * 128:
                                                lt * 512 + (i + 1) * 128],
                                         wv_t[:],
                                         start=(e == 0), stop=(e == c.NE - 1))
                for i in range(4):
                    sl = lt * 4 + i
                    nc.vector.tensor_copy(v_sb[sl][:], psv[i][:])
                    nc.scalar.dma_start(
                        cc_in[lt][(c.KV + i) * 128:(c.KV + i + 1) * 128, :],
                        v_sb[sl][:])

            def launch_ag(lt):
                nc.gpsimd.collective_compute(
                    "AllGather",
                    mybir.AluOpType.bypass,
                    replica_groups=[[2 * p, 2 * p + 1]
                                    for p in range(c.n_cores // 2)],
                    ins=[cc_in[lt].opt()],
                    outs=[cc_out[lt].opt()],
                )

            def unpack(lt):
                for h in range(c.KV):
                    nc.gpsimd.dma_start(kts[h][2 + lt][:], cc_src(lt, h))
                for i in range(4):
                    nc.gpsimd.dma_start(v_sb[8 + lt * 4 + i][:],
                                        cc_src(lt, c.KV + i))

            phase_a(0)
            launch_ag(0)
            phase_a(1)
            launch_ag(1)
            unpack(0)
            unpack(1)

            # ---------------- q-projection for one group --------------------
            def q_proj(g):
                wqt = []
                for e in range(c.NE):
                    w = wqp.tile([128, 512], BF16, tag="wq", name=f"wq{e}")
                    nc.sync.dma_start(
                        w[:], wq_d[e * 128:(e + 1) * 128,
                                   g * 512:(g + 1) * 512])
                    wqt.append(w)
                qT = []
                for blk in range(2):
                    qs = [qtp.tile([128, 512], BF16, tag="qT",
                                   name=f"qT{g}_{blk}_{jj}")
                          for jj in range(4)]
                    for hp in range(2):
                        psq = [pq.tile([128, 512], F32, tag="pq",
                                       name=f"psq{i}") for i in range(2)]
                        for e in range(c.NE):
                            for hi in range(2):
                                hh = hp * 2 + hi
                                nc.tensor.matmul(
                                    psq[hi][:],
                                    wqt[e][:, hh * 128:(hh + 1) * 128],
                                    xqT[e][:, blk * 512:(blk + 1) * 512],
                                    start=(e == 0), stop=(e == c.NE - 1))
                        for jj in range(4):
                            for hi in range(2):
                                hh = hp * 2 + hi
                                nc.vector.tensor_copy(
                                    qs[jj][:, hh * 128:(hh + 1) * 128],
                                    psq[hi][:, jj * 128:(jj + 1) * 128])
                    qT.extend(qs)
                return qT

            # ---------------- attention for one group -----------------------
            def attention(g, qT):
                for j in range(c.NJ):
                    nk = 2 * (j + 1)
                    psy = py.tile([128, 512], F32, tag="py", name="psy")
                    psums = py.tile([128, 512], F32, tag="py", name="psums")
                    for kk in range(nk):
                        sl = kk if kk <= j else 8 + (kk - j - 1)
                        sct = pa.tile([128, 512], F32, tag="pa", name="sct")
                        nc.tensor.matmul(
                            sct[:],
                            kts[g][sl // 4][:, (sl % 4) * 128:
                                            (sl % 4 + 1) * 128],
                            qT[j][:],
                            start=True, stop=True)
                        if kk == j:
                            nc.vector.tensor_add(sct[:], sct[:], masks[0][:])
                        elif kk == nk - 1:
                            nc.vector.tensor_add(sct[:], sct[:], masks[1][:])
                        pbt = smp.tile([128, 512], BF16, tag="pbt", name="pbt")
                        nc.scalar.activation(pbt[:], sct[:], AF.Exp,
                                             scale=c.scale)
                        nc.tensor.matmul(psums[:], onesb[:], pbt[:],
                                         start=(kk == 0), stop=(kk == nk - 1))
                        nc.tensor.matmul(
                            psy[:],
                            v_sb[sl][:, g * 128:(g + 1) * 128],
                            pbt[:],
                            start=(kk == 0), stop=(kk == nk - 1))
                    bsb = bsbp.tile([128, 512], F32, tag="bsb", name="bsb")
                    nc.vector.reciprocal_approx_fast(bsb[:], psums[:])
                    yt = ytp.tile([128, 512], BF16, tag="yT",
                                  name=f"yT{g}_{j}")
                    nc.vector.tensor_mul(yt[:], psy[:], bsb[:])
                    yT[g][j] = yt

            # two-group software pipeline: the PE always has a full group of
            # AG-independent q-projection work queued ahead of attention, so
            # a late exchange never stalls the in-order PE stream
            yT = [[None] * c.NJ for _ in range(c.G4)]
            qTs = [q_proj(0), q_proj(1)]
            for g in range(c.G4):
                attention(g, qTs[g])
                if g + 2 < c.G4:
                    qTs.append(q_proj(g + 2))

            # ---------------- Phase C: o-projection, single pass ------------
            for et in range(c.E // 512):
                pso = ([pq.tile([128, 512], F32, tag="pq", name=f"pso{i}")
                        for i in range(2)] +
                       [pa.tile([128, 512], F32, tag="pa", name=f"pso{i + 2}")
                        for i in range(2)] +
                       [py.tile([128, 512], F32, tag="py", name=f"pso{i + 4}")
                        for i in range(4)])
                for h in range(c.H):
                    g, hh = divmod(h, 4)
                    wo_t = wop.tile([128, 512], BF16, tag="wo", name="wo_t")
                    nc.gpsimd.dma_start(
                        wo_t[:], wo_d[h * 128:(h + 1) * 128,
                                      et * 512:(et + 1) * 512])
                    for tsub in range(c.NJ):
                        nc.tensor.matmul(
                            pso[tsub][:],
                            yT[g][tsub][:, hh * 128:(hh + 1) * 128],
                            wo_t[:],
                            start=(h == 0), stop=(h == c.H - 1))
                for tsub in range(c.NJ):
                    osb = osbp.tile([128, 512], F32, tag="osb", name="osb")
                    nc.scalar.copy(osb[:], pso[tsub][:])
                    nc.sync.dma_start(o_d[tsub * 128:(tsub + 1) * 128,
                                          et * 512:(et + 1) * 512],
                                      osb[:])

    nc.compile()
    return nc


def make_masks(cfg, s):
    """Additive causal masks in scoresT ([key, query]) orientation, tiled
    4x along the free axis for the 4-head packing.

    masks[0] is added on the own-side diagonal slot (slot j): triangular
    keep k <= q for both shards. masks[1] is added on the partner-side
    final slot (slot 8+j): for shard 0 the partner tile holds future keys
    (drop all), for shard 1 past keys (keep all).
    """
    r = np.arange(128)
    triT = np.where(r[:, None] <= r[None, :], 0.0, NEG).astype(np.float32)
    out = np.zeros((2, 128, 128), np.float32)
    out[0] = triT
    if s == 0:
        out[1] = NEG
    return np.tile(out, (1, 1, 4))


def make_inputs(cfg, x, Wq, Wk, Wv, Wo):
    """Per-core input maps from full tensors (activations/weights in bf16)."""
    bf = ml_dtypes.bfloat16
    ident_b = np.eye(128, dtype=bf)
    ones_b = np.ones((128, 128), bf)
    Wqb, Wkb, Wvb, Wob = (np.asarray(w).astype(bf) for w in (Wq, Wk, Wv, Wo))
    in_maps = []
    for cc in range(cfg.n_cores):
        b, s = divmod(cc, cfg.n_shard)
        xb = np.asarray(x[b]).astype(bf)
        xq = np.ascontiguousarray(
            xb.reshape(cfg.T // 128, 128, cfg.E)[s::cfg.n_shard]
            .reshape(cfg.RQ, cfg.E))
        poff = np.array([[((cc & 1) ^ 1) * cfg.CCB * 512]], np.uint32)
        in_maps.append({
            "xq": xq, "Wq": Wqb, "Wk": Wkb, "Wv": Wvb, "Wo": Wob,
            "masks": make_masks(cfg, s),
            "identb": ident_b,
            "onesb": ones_b,
            "poff": poff,
        })
    return in_maps


def scatter_out(cfg, results):
    B = cfg.n_batch
    out = np.empty((B, cfg.T, cfg.E), np.float32)
    for cc in range(cfg.n_cores):
        b, s = divmod(cc, cfg.n_shard)
        out[b].reshape(cfg.T // 128, 128, cfg.E)[s::cfg.n_shard] = \
            results[cc]["o"].reshape(cfg.RQ // 128, 128, cfg.E)
    return out


_NC_CACHE = {}


def get_nc(cfg):
    key = (cfg.T, cfg.E, cfg.H, cfg.KV, cfg.n_batch, cfg.n_shard)
    if key not in _NC_CACHE:
        _NC_CACHE[key] = build(cfg)
    return _NC_CACHE[key]


def run_on_hw(cfg, x, Wq, Wk, Wv, Wo, trace=False):
    nc = get_nc(cfg)
    in_maps = make_inputs(cfg, x, Wq, Wk, Wv, Wo)
    res = run_bass_kernel_spmd(nc, in_maps, list(range(cfg.n_cores)),
                               trace=trace)
    return scatter_out(cfg, [r for r in res.results]), res


def kernel(x, Wq, Wk, Wv, Wo):
    out, _ = run_on_hw(FULL, np.asarray(x), np.asarray(Wq), np.asarray(Wk),
                       np.asarray(Wv), np.asarray(Wo))
    return out


# revision 14
# speedup vs baseline: 1.1774x; 1.1774x over previous
"""GQA causal attention block (x @ Wq/Wk/Wv -> causal GQA attention -> @ Wo)
for Trainium2, SPMD over 8 NeuronCores.

Sharding: 4 batches x 2 query-shards. Core c handles batch c//2 and the
interleaved set of 128-row query tiles {s, s+2, ...} (s = c%2), which
balances the causal-attention triangle between the two shards of a batch.

vs. the v1 kernel:
- k/v are projected only for the core's own rows; the two cores of a
  batch swap halves through two 8-core AllGathers (one per 512-row
  chunk, launched as soon as that chunk's k/v is ready, running on
  TOPSP/SDMA fully overlapped with q-projection).
- x is transposed once (own rows only) and reused for q-projection.
- Wq is streamed once per head-group, Wo once; o-projection runs in a
  single pass using all 8 PSUM banks.
- softmax reciprocals use the fast approximate DVE op (~5x cheaper).

The attention inner loop keeps the dense 3-matmul form (scores,
ones-rowsum, p@V) of v1: the PE clock throttles down when its duty
cycle drops, so "saving" the rowsum matmul makes everything slower.

Key-slot layout keeps the SPMD program shard-independent: slots 0..7
hold the core's own key tiles (local order), slots 8..15 the
partner's. Query tile j attends over slots {0..j} u {8..8+j}; the
host-provided masks make it causal: masks[0] (slot j, the own-side
diagonal) is triangular for both shards, masks[1] (slot 8+j) is -inf
for shard 0 (future keys) and 0 for shard 1 (past keys). The partner
block's position in the AllGather output is the only rank-dependent
address, supplied per-core as a uint32 element offset ("poff") and
used as a runtime DMA offset register.
"""

import sys

for _p in ("/opt/trn_rl_repo", "/root/.axon_site/_ro/trn_rl_repo"):
    if _p not in sys.path:
        sys.path.append(_p)

import numpy as np
import ml_dtypes

import concourse.bacc as bacc
import concourse.bass as bass
import concourse.tile as tile
import concourse.mybir as mybir
from concourse.bass_utils import run_bass_kernel_spmd

F32 = mybir.dt.float32
BF16 = mybir.dt.bfloat16
U32 = mybir.dt.uint32
AF = mybir.ActivationFunctionType
NEG = -1.0e6  # additive mask for disallowed keys (pre-softmax-scale)


class Cfg:
    def __init__(self, T, E, H, KV, n_batch, n_shard):
        self.T, self.E, self.H, self.KV = T, E, H, KV
        self.D = 128
        self.G4 = H // 4             # 4-head kv groups
        self.NE = E // 128           # contraction chunks for projections
        self.n_batch = n_batch
        self.n_shard = n_shard
        self.n_cores = n_batch * n_shard
        self.RQ = T // n_shard       # query rows per core
        self.NJ = self.RQ // 128     # local 128-row query tiles
        self.NLT = self.RQ // 512    # local 512-row chunks
        self.NT = T // 128           # global 128-row tiles
        self.HKV = KV * self.D       # k/v projection width
        self.scale = 1.0 / float(np.sqrt(self.D))
        # per-chunk exchange block: kT (KV heads) + v (4 local tiles)
        self.CCB = (self.KV + 4) * 128  # rows per cc_in buffer


FULL = Cfg(T=2048, E=2048, H=16, KV=4, n_batch=4, n_shard=2)


def build(cfg):
    c = cfg
    nc = bacc.Bacc("TRN2", target_bir_lowering=False, debug=False,
                   num_devices=c.n_cores)

    xq_d = nc.dram_tensor("xq", [c.RQ, c.E], BF16, kind="ExternalInput").ap()
    wq_d = nc.dram_tensor("Wq", [c.E, c.H * c.D], BF16, kind="ExternalInput").ap()
    wk_d = nc.dram_tensor("Wk", [c.E, c.HKV], BF16, kind="ExternalInput").ap()
    wv_d = nc.dram_tensor("Wv", [c.E, c.HKV], BF16, kind="ExternalInput").ap()
    wo_d = nc.dram_tensor("Wo", [c.H * c.D, c.E], BF16, kind="ExternalInput").ap()
    mask_d = nc.dram_tensor("masks", [2, 128, 512], F32,
                            kind="ExternalInput").ap()
    idb_d = nc.dram_tensor("identb", [128, 128], BF16, kind="ExternalInput").ap()
    onesb_d = nc.dram_tensor("onesb", [128, 128], BF16, kind="ExternalInput").ap()
    poff_d = nc.dram_tensor("poff", [1, 1], U32, kind="ExternalInput").ap()
    o_d = nc.dram_tensor("o", [c.RQ, c.E], F32, kind="ExternalOutput").ap()

    from contextlib import ExitStack
    with tile.TileContext(nc) as tc:
        with ExitStack() as _st:
            def pool(name, bufs, space="SBUF"):
                return _st.enter_context(
                    tc.tile_pool(name=name, bufs=bufs, space=space))
            constp = pool("const", 1)
            xqtp = pool("xqt", c.NE)
            ktp = pool("kts", c.KV * 4)
            vp = pool("vsb", c.NT)
            qtp = pool("qt", 16)
            ytp = pool("yt", c.G4 * c.NJ)
            wqp = pool("wq", c.NE)
            wkvp = pool("wkv", 6)
            wop = pool("wo", 8)
            smp = pool("sm", 8)
            bsbp = pool("bsb", 4)
            xnp = pool("xn", 8)
            osbp = pool("osb", 6)
            pq = pool("pq", 2, space="PSUM")
            pa = pool("pa", 2, space="PSUM")
            py = pool("py", 4, space="PSUM")
            dramp = pool("dram", 1, space="DRAM")

            # --- constants (identb first: the warmup needs it; the rest go
            # on the scalar queue so they don't delay the first x tiles) ---
            identb = constp.tile([128, 128], BF16, tag="identb")
            nc.sync.dma_start(identb[:], idb_d[:])
            masks = []
            for i in range(2):
                m = constp.tile([128, 512], F32, tag=f"mask{i}", name=f"mask{i}")
                nc.sync.dma_start(m[:], mask_d[i])
                masks.append(m)
            onesb = constp.tile([128, 128], BF16, tag="onesb")
            nc.sync.dma_start(onesb[:], onesb_d[:])
            poffs = constp.tile([1, 1], U32, tag="poffs")
            nc.sync.dma_start(poffs[:], poff_d[:])

            cc_in = [dramp.tile([c.CCB, 512], BF16, name=f"cc_in{lt}",
                                tag=f"cc_in{lt}") for lt in range(c.NLT)]
            cc_out = [dramp.tile([2 * c.CCB, 512], BF16,
                                 name=f"cc_out{lt}",
                                 tag=f"cc_out{lt}") for lt in range(c.NLT)]

            # warm the PE clock-gate during the initial DMA ramp
            pwu = pa.tile([128, 512], BF16, tag="pa", name="pwu")
            for wu in range(24):
                nc.tensor.transpose(pwu[:, (wu % 4) * 128:(wu % 4 + 1) * 128],
                                    identb[:], identb[:])

            # persistent activations
            xqT = [xqtp.tile([128, c.RQ], BF16, tag="xqT", name=f"xqT{e}")
                   for e in range(c.NE)]
            kts = [[ktp.tile([128, 512], BF16, tag="kts", name=f"kts{h}_{q}")
                    for q in range(4)] for h in range(c.KV)]
            v_sb = [vp.tile([128, c.HKV], BF16, tag="v", name=f"v{i}")
                    for i in range(c.NT)]

            # partner block offset (elements) comes from host data
            poff_r = nc.gpsimd.alloc_register("poff_r")
            nc.gpsimd.reg_load(poff_r, poffs[0:1, 0:1])
            poff_v = nc.gpsimd.snap(poff_r, donate=True, min_val=0,
                                    max_val=c.CCB * 512)

            def cc_src(lt, block):
                off = poff_v + block * 128 * 512
                return bass.AP(cc_out[lt].tensor, off, [[512, 128], [1, 512]])

            # ---------------- Phase A: transposes + own-half k/v ------------
            def phase_a(lt):
                # transpose own 512 rows into xqT[e][:, lt*512:(lt+1)*512]
                for qa in range(c.NE // 4):
                    xns = []
                    for i in range(4):
                        xn = xnp.tile([128, 512], BF16, tag="xn",
                                      name=f"xn{i}")
                        nc.sync.dma_start(
                            xn[:], xq_d[lt * 512 + i * 128:
                                        lt * 512 + (i + 1) * 128,
                                        qa * 512:(qa + 1) * 512])
                        xns.append(xn)
                    for eh in range(4):
                        e = qa * 4 + eh
                        ptr = pa.tile([128, 512], BF16, tag="pa", name="ptr")
                        for i in range(4):
                            nc.tensor.transpose(
                                ptr[:, i * 128:(i + 1) * 128],
                                xns[i][:, eh * 128:(eh + 1) * 128], identb[:])
                        nc.vector.tensor_copy(
                            xqT[e][:, lt * 512:(lt + 1) * 512], ptr[:])

                # kT for own rows -> slots 4*lt..4*lt+3 (= quad lt)
                psk = ([pq.tile([128, 512], F32, tag="pq", name=f"psk{h}")
                        for h in range(2)] +
                       [pa.tile([128, 512], F32, tag="pa", name=f"psk{h + 2}")
                        for h in range(2)])
                for e in range(c.NE):
                    wk_t = wkvp.tile([128, c.HKV], BF16, tag="wkv", name="wk_t")
                    nc.gpsimd.dma_start(wk_t[:], wk_d[e * 128:(e + 1) * 128, :])
                    for h in range(c.KV):
                        nc.tensor.matmul(psk[h][:],
                                         wk_t[:, h * 128:(h + 1) * 128],
                                         xqT[e][:, lt * 512:(lt + 1) * 512],
                                         start=(e == 0), stop=(e == c.NE - 1))
                for h in range(c.KV):
                    nc.vector.tensor_copy(kts[h][lt][:], psk[h][:])
                    nc.gpsimd.dma_start(
                        cc_in[lt][h * 128:(h + 1) * 128, :], kts[h][lt][:])

                # v for own rows -> slots 4*lt..4*lt+3
                psv = ([pq.tile([128, c.HKV], F32, tag="pq", name=f"psv{i}")
                        for i in range(2)] +
                       [pa.tile([128, c.HKV], F32, tag="pa", name=f"psv{i + 2}")
                        for i in range(2)])
                for e in range(c.NE):
                    wv_t = wkvp.tile([128, c.HKV], BF16, tag="wkv", name="wv_t")
                    nc.gpsimd.dma_start(wv_t[:], wv_d[e * 128:(e + 1) * 128, :])
                    for i in range(4):
                        nc.tensor.matmul(psv[i][:],
                                         xqT[e][:, lt * 512 + i * 128:
                                                lt * 512 + (i + 1) * 128],
                                         wv_t[:],
                                         start=(e == 0), stop=(e == c.NE - 1))
                for i in range(4):
                    sl = lt * 4 + i
                    nc.vector.tensor_copy(v_sb[sl][:], psv[i][:])
                    nc.gpsimd.dma_start(
                        cc_in[lt][(c.KV + i) * 128:(c.KV + i + 1) * 128, :],
                        v_sb[sl][:])

            def launch_ag(lt):
                nc.gpsimd.collective_compute(
                    "AllGather",
                    mybir.AluOpType.bypass,
                    replica_groups=[[2 * p, 2 * p + 1]
                                    for p in range(c.n_cores // 2)],
                    ins=[cc_in[lt].opt()],
                    outs=[cc_out[lt].opt()],
                )

            def unpack(lt):
                for h in range(c.KV):
                    nc.gpsimd.dma_start(kts[h][2 + lt][:], cc_src(lt, h))
                for i in range(4):
                    nc.gpsimd.dma_start(v_sb[8 + lt * 4 + i][:],
                                        cc_src(lt, c.KV + i))

            phase_a(0)
            launch_ag(0)
            phase_a(1)
            launch_ag(1)
            unpack(0)
            unpack(1)

            # ---------------- q-projection for one group --------------------
            def q_proj(g):
                wqt = []
                for e in range(c.NE):
                    w = wqp.tile([128, 512], BF16, tag="wq", name=f"wq{e}")
                    nc.sync.dma_start(
                        w[:], wq_d[e * 128:(e + 1) * 128,
                                   g * 512:(g + 1) * 512])
                    wqt.append(w)
                qT = []
                for blk in range(2):
                    qs = [qtp.tile([128, 512], BF16, tag="qT",
                                   name=f"qT{g}_{blk}_{jj}")
                          for jj in range(4)]
                    for hp in range(2):
                        psq = [pq.tile([128, 512], F32, tag="pq",
                                       name=f"psq{i}") for i in range(2)]
                        for e in range(c.NE):
                            for hi in range(2):
                                hh = hp * 2 + hi
                                nc.tensor.matmul(
                                    psq[hi][:],
                                    wqt[e][:, hh * 128:(hh + 1) * 128],
                                    xqT[e][:, blk * 512:(blk + 1) * 512],
                                    start=(e == 0), stop=(e == c.NE - 1))
                        for jj in range(4):
                            for hi in range(2):
                                hh = hp * 2 + hi
                                nc.vector.tensor_copy(
                                    qs[jj][:, hh * 128:(hh + 1) * 128],
                                    psq[hi][:, jj * 128:(jj + 1) * 128])
                    qT.extend(qs)
                return qT

            # ---------------- attention for one group -----------------------
            def attention(g, qT):
                for j in range(c.NJ):
                    nk = 2 * (j + 1)
                    psy = py.tile([128, 512], F32, tag="py", name="psy")
                    psums = py.tile([128, 512], F32, tag="py", name="psums")
                    for kk in range(nk):
                        sl = kk if kk <= j else 8 + (kk - j - 1)
                        sct = pa.tile([128, 512], F32, tag="pa", name="sct")
                        nc.tensor.matmul(
                            sct[:],
                            kts[g][sl // 4][:, (sl % 4) * 128:
                                            (sl % 4 + 1) * 128],
                            qT[j][:],
                            start=True, stop=True)
                        if kk == j:
                            nc.vector.tensor_add(sct[:], sct[:], masks[0][:])
                        elif kk == nk - 1:
                            nc.vector.tensor_add(sct[:], sct[:], masks[1][:])
                        pbt = smp.tile([128, 512], BF16, tag="pbt", name="pbt")
                        nc.scalar.activation(pbt[:], sct[:], AF.Exp,
                                             scale=c.scale)
                        nc.tensor.matmul(psums[:], onesb[:], pbt[:],
                                         start=(kk == 0), stop=(kk == nk - 1))
                        nc.tensor.matmul(
                            psy[:],
                            v_sb[sl][:, g * 128:(g + 1) * 128],
                            pbt[:],
                            start=(kk == 0), stop=(kk == nk - 1))
                    bsb = bsbp.tile([128, 512], F32, tag="bsb", name="bsb")
                    nc.vector.reciprocal_approx_fast(bsb[:], psums[:])
                    yt = ytp.tile([128, 512], BF16, tag="yT",
                                  name=f"yT{g}_{j}")
                    nc.vector.tensor_mul(yt[:], psy[:], bsb[:])
                    yT[g][j] = yt

            # two-group software pipeline: the PE always has a full group of
            # AG-independent q-projection work queued ahead of attention, so
            # a late exchange never stalls the in-order PE stream
            yT = [[None] * c.NJ for _ in range(c.G4)]
            qTs = [q_proj(0), q_proj(1)]
            for g in range(c.G4):
                attention(g, qTs[g])
                if g + 2 < c.G4:
                    qTs.append(q_proj(g + 2))

            # ---------------- Phase C: o-projection, single pass ------------
            for et in range(c.E // 512):
                pso = ([pq.tile([128, 512], F32, tag="pq", name=f"pso{i}")
                        for i in range(2)] +
                       [pa.tile([128, 512], F32, tag="pa", name=f"pso{i + 2}")
                        for i in range(2)] +
                       [py.tile([128, 512], F32, tag="py", name=f"pso{i + 4}")
                        for i in range(4)])
                for h in range(c.H):
                    g, hh = divmod(h, 4)
                    wo_t = wop.tile([128, 512], BF16, tag="wo", name="wo_t")
                    nc.gpsimd.dma_start(
                        wo_t[:], wo_d[h * 128:(h + 1) * 128,
                                      et * 512:(et + 1) * 512])
                    for tsub in range(c.NJ):
                        nc.tensor.matmul(
                            pso[tsub][:],
                            yT[g][tsub][:, hh * 128:(hh + 1) * 128],
                            wo_t[:],
                            start=(h == 0), stop=(h == c.H - 1))
                for tsub in range(c.NJ):
                    osb = osbp.tile([128, 512], F32, tag="osb", name="osb")
                    nc.scalar.copy(osb[:], pso[tsub][:])
                    nc.sync.dma_start(o_d[tsub * 128:(tsub + 1) * 128,
                                          et * 512:(et + 1) * 512],
                                      osb[:])

    nc.compile()
    return nc


def make_masks(cfg, s):
    """Additive causal masks in scoresT ([key, query]) orientation, tiled
    4x along the free axis for the 4-head packing.

    masks[0] is added on the own-side diagonal slot (slot j): triangular
    keep k <= q for both shards. masks[1] is added on the partner-side
    final slot (slot 8+j): for shard 0 the partner tile holds future keys
    (drop all), for shard 1 past keys (keep all).
    """
    r = np.arange(128)
    triT = np.where(r[:, None] <= r[None, :], 0.0, NEG).astype(np.float32)
    out = np.zeros((2, 128, 128), np.float32)
    out[0] = triT
    if s == 0:
        out[1] = NEG
    return np.tile(out, (1, 1, 4))


def make_inputs(cfg, x, Wq, Wk, Wv, Wo):
    """Per-core input maps from full tensors (activations/weights in bf16)."""
    bf = ml_dtypes.bfloat16
    ident_b = np.eye(128, dtype=bf)
    ones_b = np.ones((128, 128), bf)
    Wqb, Wkb, Wvb, Wob = (np.asarray(w).astype(bf) for w in (Wq, Wk, Wv, Wo))
    in_maps = []
    for cc in range(cfg.n_cores):
        b, s = divmod(cc, cfg.n_shard)
        xb = np.asarray(x[b]).astype(bf)
        xq = np.ascontiguousarray(
            xb.reshape(cfg.T // 128, 128, cfg.E)[s::cfg.n_shard]
            .reshape(cfg.RQ, cfg.E))
        poff = np.array([[((cc & 1) ^ 1) * cfg.CCB * 512]], np.uint32)
        in_maps.append({
            "xq": xq, "Wq": Wqb, "Wk": Wkb, "Wv": Wvb, "Wo": Wob,
            "masks": make_masks(cfg, s),
            "identb": ident_b,
            "onesb": ones_b,
            "poff": poff,
        })
    return in_maps


def scatter_out(cfg, results):
    B = cfg.n_batch
    out = np.empty((B, cfg.T, cfg.E), np.float32)
    for cc in range(cfg.n_cores):
        b, s = divmod(cc, cfg.n_shard)
        out[b].reshape(cfg.T // 128, 128, cfg.E)[s::cfg.n_shard] = \
            results[cc]["o"].reshape(cfg.RQ // 128, 128, cfg.E)
    return out


_NC_CACHE = {}


def get_nc(cfg):
    key = (cfg.T, cfg.E, cfg.H, cfg.KV, cfg.n_batch, cfg.n_shard)
    if key not in _NC_CACHE:
        _NC_CACHE[key] = build(cfg)
    return _NC_CACHE[key]


def run_on_hw(cfg, x, Wq, Wk, Wv, Wo, trace=False):
    nc = get_nc(cfg)
    in_maps = make_inputs(cfg, x, Wq, Wk, Wv, Wo)
    res = run_bass_kernel_spmd(nc, in_maps, list(range(cfg.n_cores)),
                               trace=trace)
    return scatter_out(cfg, [r for r in res.results]), res


def kernel(x, Wq, Wk, Wv, Wo):
    out, _ = run_on_hw(FULL, np.asarray(x), np.asarray(Wq), np.asarray(Wk),
                       np.asarray(Wv), np.asarray(Wo))
    return out
